# revision 18
# baseline (speedup 1.0000x reference)
"""Trainium2 Bass kernel for nn_AbstractiveTextSummarize (pointer-generator
hierarchical summarizer). Batch-parallel over 8 NeuronCores: core k runs the
full model for batch element k (encoder BiGRUs + 32-step attention decoder +
32k-vocab projection/softmax + pointer scatter-add via SDMA CCE).

Self-contained: hardcodes all shapes from the problem spec.
Layout: activations transposed (feature-on-partition) so matmuls contract
over partitions. 2D SBUF tiles [128, ntiles*w]; k-tile j = cols [j*w:(j+1)*w].
fp32 everywhere except the 32k projection (bf16). Only Tanh/Exp activations
-> single ACT table set (exp_and_others), no reload inside loops.
"""
import numpy as np
import ml_dtypes

import concourse.bass as bass
from concourse import bacc
import concourse.mybir as mybir
from concourse.tile import TileContext
from concourse.bass_utils import run_bass_kernel_spmd

F32 = mybir.dt.float32
BF16 = mybir.dt.bfloat16
I16 = mybir.dt.int16

V = 32000
NOOV = 100
VEXT = V + NOOV          # 32100
VPAD = 32256             # 63 * 512
H = 256
D = 512
B, S, W, ST = 8, 16, 32, 32
NPOS = S * W             # 512
BOS = 1
NCH = VPAD // 512        # 63

ACT = mybir.ActivationFunctionType
ALU = mybir.AluOpType
AX = mybir.AxisListType

LAST_RESULTS = None


def build_nc():
    nc = bacc.Bacc(None)
    # register 0.5 as a const AP (used by activation bias for sigmoid-via-tanh)
    _c = nc.alloc_sbuf_tensor("const-float32-0.5", [128, 1], F32)
    nc.gpsimd.memset(_c.ap(), 0.5)
    nc.const_aps.aps[(F32, 0.5)] = _c.ap()
    nc.all_engine_barrier()
    dp = lambda n, sh, dt=F32: nc.declare_dram_parameter(n, list(sh), dt, isOutput=False)

    xTa_d  = dp("xTa",  [2, 128, NPOS])
    wWf_d  = dp("wWf",  [2, 128, 768]); wWb_d = dp("wWb", [2, 128, 768])
    wUf_d  = dp("wUf",  [2, 128, 768]); wUb_d = dp("wUb", [2, 128, 768])
    sWf_d  = dp("sWf",  [4, 128, 768]); sWb_d = dp("sWb", [4, 128, 768])
    sBf_d  = dp("sBf",  [1, 768]);      sBb_d = dp("sBb", [1, 768])
    sUf_d  = dp("sUf",  [2, 128, 768]); sUb_d = dp("sUb", [2, 128, 768])
    waW_d  = dp("waW",  [128, 4])
    aW1T_d = dp("aW1T", [4, 128, 512]); aW2T_d = dp("aW2T", [4, 128, 512])
    sW1T_d = dp("sW1T", [4, 128, 512]); sW2T_d = dp("sW2T", [4, 128, 512])
    aC_d   = dp("aC",   [128, 4]);      sC_d  = dp("sC", [128, 4])
    awV_d  = dp("awV",  [128, 4]);      asV_d = dp("asV", [128, 4])
    Esb_d  = dp("Esb",  [16, NPOS])
    speT_d = dp("speT", [4, 128, 16])
    OT_d   = dp("OT",   [4, 128, ST])
    dWo_d  = dp("dWo",  [4, 128, 1536])
    dWc_d  = dp("dWc",  [4, 128, 1536])
    dU_d   = dp("dU",   [4, 128, 1536])
    dB_d   = dp("dB",   [1, 1536])
    selWT_d = dp("selWT", [12, 128, 512])
    selB_d = dp("selB", [1, 512])
    vsW_d  = dp("vsW",  [1, 512])
    vsb_d  = dp("vsb",  [1, 1])
    G_d    = dp("G",    [4, 128, 512])
    SIDX_d = dp("SIDX", [128, 32], I16)
    oWT_d  = dp("oWT",  [4, 128, VPAD], BF16)
    oB_d   = dp("oB",   [1, VPAD], BF16)
    I128_d = dp("I128", [128, 128])

    dense_d = nc.declare_dram_parameter("dense", [ST, V], F32, isOutput=True)
    scat_d  = nc.declare_dram_parameter("scat", [VEXT, 64], F32, isOutput=True)

    with TileContext(nc) as tc, \
         tc.tile_pool(name="per", bufs=1) as per, \
         tc.tile_pool(name="psA", bufs=3, space="PSUM") as psA, \
         tc.tile_pool(name="psB", bufs=3, space="PSUM") as psB, \
         tc.tile_pool(name="psC", bufs=1, space="PSUM") as psC:

        def load(pool, d, kt, cols, dtype=F32, name=None):
            tg = name or f"ld_{d.name}"
            t = pool.tile([128, kt * cols], dtype, name=name, tag=tg)
            for k in range(kt):
                nc.sync.dma_start(out=t[:, k * cols:(k + 1) * cols], in_=d[k])
            return t

        def load2(pool, d, shape, dtype=F32, name=None):
            tg = name or f"ld_{d.name}"
            t = pool.tile(list(shape), dtype, name=name, tag=tg)
            nc.sync.dma_start(out=t[:], in_=d[:])
            return t

        I128 = load2(per, I128_d, [128, 128], name="I128")
        ones1 = per.tile([1, 1], F32, name="ones1"); nc.vector.memset(ones1[:], 1.0)
        ones32 = per.tile([1, 32], F32, name="ones32"); nc.vector.memset(ones32[:], 1.0)

        def pe_t(src_ap, p, f):
            o = psA.tile([f, p], F32, tag="a")
            nc.tensor.transpose(o[:], src_ap, I128[0:p, 0:p])
            return o

        whT = per.tile([128, 4 * NPOS], F32, name="whT")
        whR = per.tile([128, 4 * D], F32, name="whR")
        wpT = per.tile([128, 4 * NPOS], F32, name="wpT")
        shT = per.tile([128, 4 * S], F32, name="shT")
        spT = per.tile([128, 4 * S], F32, name="spT")
        OTs = per.tile([128, 4 * ST], F32, name="OTs")
        H2T = per.tile([128, 4 * ST], F32, name="H2T")
        PAT = per.tile([128, 4 * 64], F32, name="PAT")   # (1-pg)*pa^T, col t
        PGf = per.tile([1, ST], F32, name="PGf")
        h0T = per.tile([128, 4], F32, name="h0T")
        ones128r = per.tile([1, 128], F32, name="ones128r")
        nc.vector.memset(ones128r[:], 1.0)
        nc.vector.memset(PAT[:], 0.0)

        for k in range(4):
            nc.sync.dma_start(out=OTs[:, k * ST:(k + 1) * ST], in_=OT_d[k])

        # ==================== ENCODER ====================
        with tc.tile_pool(name="enc", bufs=1) as enc:
            xTa = load(enc, xTa_d, 2, NPOS, name="xTa")
            wW = {0: load(enc, wWf_d, 2, 768), 1: load(enc, wWb_d, 2, 768)}
            wU = {0: load(enc, wUf_d, 2, 768), 1: load(enc, wUb_d, 2, 768)}
            waW = load2(enc, waW_d, [128, 4])
            Esb = load2(enc, Esb_d, [16, NPOS])
            speT = load(enc, speT_d, 4, 16)
            sWm = {0: load(enc, sWf_d, 4, 768), 1: load(enc, sWb_d, 4, 768)}
            sB = {0: load2(enc, sBf_d, [1, 768]), 1: load2(enc, sBb_d, [1, 768])}
            sU = {0: load(enc, sUf_d, 2, 768), 1: load(enc, sUb_d, 2, 768)}
            aW1T = load(enc, aW1T_d, 4, 512)
            sW1T = load(enc, sW1T_d, 4, 512)

            # E1: word-GRU input gates, both dirs: giT [768, 512tok], w-major tokens
            giT = {}
            for dr in (0, 1):
                g = enc.tile([128, 6 * NPOS], F32, name=f"giT{dr}")
                giT[dr] = g
                for m in range(6):
                    ps = psB.tile([128, NPOS], F32, tag="b")
                    for k in range(2):
                        nc.tensor.matmul(
                            ps[:], wW[dr][:, k * 768 + m * 128: k * 768 + (m + 1) * 128],
                            xTa[:, k * NPOS:(k + 1) * NPOS],
                            start=(k == 0), stop=(k == 1))
                    nc.vector.tensor_copy(g[:, m * NPOS:(m + 1) * NPOS], ps[:])

            # E2: word BiGRU, state [128, 2*16]
            hst = {}
            for dr in (0, 1):
                hh = enc.tile([128, 32], F32, name=f"hw{dr}")
                nc.vector.memset(hh[:], 0.0)
                hst[dr] = hh
            giv = {dr: giT[dr][:].rearrange("p (m n) -> p m n", m=6) for dr in (0, 1)}
            whTw = whT[:].rearrange("p (k s w) -> p w k s", k=4, s=S)

            def wgru_step(dr, i):
                w_in = i if dr == 0 else (W - 1 - i)
                hh = hst[dr]
                gh = psB.tile([128, 96], F32, tag="b", name="ghw")
                for m in range(6):
                    for k in range(2):
                        nc.tensor.matmul(
                            gh[:, m * 16:(m + 1) * 16],
                            wU[dr][:, k * 768 + m * 128: k * 768 + (m + 1) * 128],
                            hh[:, k * 16:(k + 1) * 16],
                            start=(k == 0), stop=(k == 1))
                gi = giv[dr]
                rzp = enc.tile([128, 64], F32, name="rzp", bufs=2)
                nc.vector.tensor_add(
                    rzp[:].rearrange("p (m n) -> p m n", m=4),
                    gi[:, 0:4, w_in * 16:(w_in + 1) * 16],
                    gh[:, 0:64].rearrange("p (m n) -> p m n", m=4))
                rz = enc.tile([128, 64], F32, name="rz", bufs=2)
                nc.scalar.activation(rz[:], rzp[:], ACT.Tanh, scale=0.5)
                nc.scalar.activation(rz[:], rz[:], ACT.Identity, bias=0.5, scale=0.5)
                npre = enc.tile([128, 32], F32, name="npre", bufs=2)
                nc.vector.tensor_mul(npre[:], rz[:, 0:32], gh[:, 64:96])
                nc.vector.tensor_add(
                    npre[:].rearrange("p (m n) -> p m n", m=2),
                    npre[:].rearrange("p (m n) -> p m n", m=2),
                    gi[:, 4:6, w_in * 16:(w_in + 1) * 16])
                nn_ = enc.tile([128, 32], F32, name="nn", bufs=2)
                nc.scalar.activation(nn_[:], npre[:], ACT.Tanh)
                dd = enc.tile([128, 32], F32, name="dd", bufs=2)
                nc.vector.tensor_sub(dd[:], hh[:], nn_[:])
                nc.vector.tensor_mul(dd[:], rz[:, 32:64], dd[:])
                nc.vector.tensor_add(hh[:], nn_[:], dd[:])
                kofs = 0 if dr == 0 else 2
                nc.vector.tensor_copy(
                    whTw[:, w_in:w_in + 1, kofs:kofs + 2, :],
                    hh[:].rearrange("p (o k s) -> p o k s", o=1, k=2))

            for i in range(W):
                wgru_step(0, i)
                wgru_step(1, i)

            for k in range(2):
                nc.vector.tensor_copy(h0T[:, k:k + 1], hst[0][:, k * 16 + 15: k * 16 + 16])
                nc.vector.tensor_copy(h0T[:, 2 + k:3 + k], hst[1][:, k * 16 + 15: k * 16 + 16])

            # E3a: wh row-major
            for i in range(4):
                for j in range(4):
                    ps = pe_t(whT[:, j * NPOS + i * 128: j * NPOS + (i + 1) * 128], 128, 128)
                    nc.vector.tensor_copy(whR[:, i * D + j * 128: i * D + (j + 1) * 128], ps[:])

            # E3b: word attention -> sent_vec^T
            scw = psB.tile([1, NPOS], F32, tag="b", name="scw")
            for k in range(4):
                nc.tensor.matmul(scw[:], waW[:, k:k + 1], whT[:, k * NPOS:(k + 1) * NPOS],
                                 start=(k == 0), stop=(k == 3))
            ew = enc.tile([1, NPOS], F32, name="ew")
            nc.scalar.activation(ew[:], scw[:], ACT.Exp)
            sw = enc.tile([1, S], F32, name="sw")
            nc.vector.tensor_reduce(sw[:], ew[:].rearrange("p (s w) -> p s w", s=S),
                                    axis=AX.X, op=ALU.add)
            rw = enc.tile([1, S], F32, name="rw")
            nc.vector.reciprocal(rw[:], sw[:])
            fT = pe_t(rw[:], 1, 16)
            fTs = enc.tile([16, 1], F32, name="fTs")
            nc.vector.tensor_copy(fTs[:], fT[:])
            ffull = psB.tile([1, NPOS], F32, tag="b", name="ffull")
            nc.tensor.matmul(ffull[:], fTs[:], Esb[:], start=True, stop=True)
            aw = enc.tile([1, NPOS], F32, name="aw")
            nc.vector.tensor_mul(aw[:], ew[:], ffull[:])
            AWm = enc.tile([128, 4 * 16], F32, name="AWm")
            nc.vector.memset(AWm[:], 0.0)
            for k in range(4):
                ps = pe_t(aw[0:1, k * 128:(k + 1) * 128], 1, 128)
                for q in range(4):
                    s = k * 4 + q
                    nc.vector.tensor_copy(
                        AWm[q * 32:(q + 1) * 32, k * 16 + s: k * 16 + s + 1],
                        ps[q * 32:(q + 1) * 32, :])
            svT = enc.tile([128, 4 * 16], F32, name="svT")
            for m in range(4):
                ps = psA.tile([128, 16], F32, tag="a")
                for k in range(4):
                    nc.tensor.matmul(ps[:], whR[:, k * D + m * 128: k * D + (m + 1) * 128],
                                     AWm[:, k * 16:(k + 1) * 16],
                                     start=(k == 0), stop=(k == 3))
                nc.vector.tensor_add(svT[:, m * 16:(m + 1) * 16], ps[:],
                                     speT[:, m * 16:(m + 1) * 16])

            # E4: sentence BiGRU
            gisT = {}
            for dr in (0, 1):
                g = enc.tile([128, 6 * S], F32, name=f"gisT{dr}")
                gisT[dr] = g
                for m in range(6):
                    ps = psA.tile([128, S], F32, tag="a")
                    for k in range(4):
                        nc.tensor.matmul(
                            ps[:], sWm[dr][:, k * 768 + m * 128: k * 768 + (m + 1) * 128],
                            svT[:, k * 16:(k + 1) * 16], start=(k == 0), stop=False)
                    nc.tensor.matmul(ps[:], sB[dr][0:1, m * 128:(m + 1) * 128],
                                     ones32[0:1, 0:16], start=False, stop=True)
                    nc.vector.tensor_copy(g[:, m * S:(m + 1) * S], ps[:])
            hs = {}
            for dr in (0, 1):
                t = enc.tile([128, 2], F32, name=f"hs{dr}")
                nc.vector.memset(t[:], 0.0)
                hs[dr] = t
            gisv = {dr: gisT[dr][:].rearrange("p (m n) -> p m n", m=6) for dr in (0, 1)}
            shT3 = shT[:].rearrange("p (k s) -> p k s", k=4)

            def sgru_step(dr, i):
                s_in = i if dr == 0 else (S - 1 - i)
                hh = hs[dr]
                gh = psA.tile([128, 6], F32, tag="a", name="ghs")
                for m in range(6):
                    for k in range(2):
                        nc.tensor.matmul(
                            gh[:, m:m + 1],
                            sU[dr][:, k * 768 + m * 128: k * 768 + (m + 1) * 128],
                            hh[:, k:k + 1], start=(k == 0), stop=(k == 1))
                gi = gisv[dr]
                rzp = enc.tile([128, 4], F32, name="srzp", bufs=2)
                nc.vector.tensor_add(
                    rzp[:].rearrange("p (m o) -> p m o", m=4),
                    gi[:, 0:4, s_in:s_in + 1],
                    gh[:, 0:4].rearrange("p (m o) -> p m o", m=4))
                rz = enc.tile([128, 4], F32, name="srz", bufs=2)
                nc.scalar.activation(rz[:], rzp[:], ACT.Tanh, scale=0.5)
                nc.scalar.activation(rz[:], rz[:], ACT.Identity, bias=0.5, scale=0.5)
                npre = enc.tile([128, 2], F32, name="snpre", bufs=2)
                nc.vector.tensor_mul(npre[:], rz[:, 0:2], gh[:, 4:6])
                nc.vector.tensor_add(
                    npre[:].rearrange("p (m o) -> p m o", m=2),
                    npre[:].rearrange("p (m o) -> p m o", m=2),
                    gi[:, 4:6, s_in:s_in + 1])
                nn_ = enc.tile([128, 2], F32, name="snn", bufs=2)
                nc.scalar.activation(nn_[:], npre[:], ACT.Tanh)
                dd = enc.tile([128, 2], F32, name="sdd", bufs=2)
                nc.vector.tensor_sub(dd[:], hh[:], nn_[:])
                nc.vector.tensor_mul(dd[:], rz[:, 2:4], dd[:])
                nc.vector.tensor_add(hh[:], nn_[:], dd[:])
                kofs = 0 if dr == 0 else 2
                nc.vector.tensor_copy(
                    shT3[:, kofs:kofs + 2, s_in:s_in + 1],
                    hh[:].rearrange("p (k o) -> p k o", k=2))

            for i in range(S):
                sgru_step(0, i)
                sgru_step(1, i)

            # E5: wpT = aw_W1 @ whT ; spT = as_W1 @ shT
            for m in range(4):
                ps = psB.tile([128, NPOS], F32, tag="b")
                for k in range(4):
                    nc.tensor.matmul(ps[:], aW1T[:, k * 512 + m * 128: k * 512 + (m + 1) * 128],
                                     whT[:, k * NPOS:(k + 1) * NPOS],
                                     start=(k == 0), stop=(k == 3))
                nc.vector.tensor_copy(wpT[:, m * NPOS:(m + 1) * NPOS], ps[:])
                ps2 = psA.tile([128, S], F32, tag="a")
                for k in range(4):
                    nc.tensor.matmul(ps2[:], sW1T[:, k * 512 + m * 128: k * 512 + (m + 1) * 128],
                                     shT[:, k * S:(k + 1) * S],
                                     start=(k == 0), stop=(k == 3))
                nc.vector.tensor_copy(spT[:, m * S:(m + 1) * S], ps2[:])

        # ==================== DECODER phase A ====================
        with tc.tile_pool(name="dec", bufs=1) as dec:
            aW2T = load(dec, aW2T_d, 4, 512)
            sW2T = load(dec, sW2T_d, 4, 512)
            aC = load2(dec, aC_d, [128, 4]); sC = load2(dec, sC_d, [128, 4])
            awV = load2(dec, awV_d, [128, 4]); asV = load2(dec, asV_d, [128, 4])
            Esb2 = load2(dec, Esb_d, [16, NPOS], name="Esb2")
            dWc = load(dec, dWc_d, 4, 1536)
            dU = load(dec, dU_d, 4, 1536)
            selWT = load(dec, selWT_d, 12, 512)
            selB = load2(dec, selB_d, [1, 512])
            vsW = load2(dec, vsW_d, [1, 512])
            vsb = load2(dec, vsb_d, [1, 1])

            gioT = dec.tile([128, 12 * ST], F32, name="gioT")
            with tc.tile_pool(name="dwo", bufs=1) as dwo:
                dWo = load(dwo, dWo_d, 4, 1536)
                dBt = load2(dwo, dB_d, [1, 1536])
                for m in range(12):
                    ps = psA.tile([128, ST], F32, tag="a")
                    for k in range(4):
                        nc.tensor.matmul(
                            ps[:], dWo[:, k * 1536 + m * 128: k * 1536 + (m + 1) * 128],
                            OTs[:, k * ST:(k + 1) * ST], start=(k == 0), stop=False)
                    nc.tensor.matmul(ps[:], dBt[0:1, m * 128:(m + 1) * 128], ones32[:],
                                     start=False, stop=True)
                    nc.vector.tensor_copy(gioT[:, m * ST:(m + 1) * ST], ps[:])

            hT = dec.tile([128, 4], F32, name="hT")
            nc.vector.tensor_copy(hT[:], h0T[:])
            giov = gioT[:].rearrange("p (m t) -> p m t", m=12)
            H2T3 = H2T[:].rearrange("p (k t) -> p k t", k=4)

            for t in range(ST):
                # gh half of the cell: gH [128,12] = Whh^T-contract with hT
                gH = psC.tile([128, 12], F32, tag="c", name="gH")
                for m in range(12):
                    for k in range(4):
                        nc.tensor.matmul(
                            gH[:, m:m + 1],
                            dU[:, k * 1536 + m * 128: k * 1536 + (m + 1) * 128],
                            hT[:, k:k + 1], start=(k == 0), stop=(k == 3))
                # attention queries
                dpw = psA.tile([128, 4], F32, tag="a", name="dpw")
                dps = psA.tile([128, 4], F32, tag="a", name="dps")
                for m in range(4):
                    for k in range(4):
                        nc.tensor.matmul(dpw[:, m:m + 1],
                                         aW2T[:, k * 512 + m * 128: k * 512 + (m + 1) * 128],
                                         hT[:, k:k + 1], start=(k == 0), stop=(k == 3))
                        nc.tensor.matmul(dps[:, m:m + 1],
                                         sW2T[:, k * 512 + m * 128: k * 512 + (m + 1) * 128],
                                         hT[:, k:k + 1], start=(k == 0), stop=(k == 3))
                bw = dec.tile([128, 4], F32, name="bw", bufs=2)
                bs = dec.tile([128, 4], F32, name="bs", bufs=2)
                nc.vector.tensor_add(bw[:], dpw[:], aC[:])
                nc.vector.tensor_add(bs[:], dps[:], sC[:])
                tw = dec.tile([128, 4 * NPOS], F32, name="tw", bufs=2)
                ts = dec.tile([128, 4 * S], F32, name="ts", bufs=2)
                for k in range(4):
                    nc.scalar.activation(tw[:, k * NPOS:(k + 1) * NPOS],
                                         wpT[:, k * NPOS:(k + 1) * NPOS],
                                         ACT.Tanh, bias=bw[:, k:k + 1])
                    nc.scalar.activation(ts[:, k * S:(k + 1) * S],
                                         spT[:, k * S:(k + 1) * S],
                                         ACT.Tanh, bias=bs[:, k:k + 1])
                scw = psB.tile([1, NPOS], F32, tag="b", name="dscw")
                scs = psA.tile([1, S], F32, tag="a", name="dscs")
                for k in range(4):
                    nc.tensor.matmul(scw[:], awV[:, k:k + 1], tw[:, k * NPOS:(k + 1) * NPOS],
                                     start=(k == 0), stop=(k == 3))
                    nc.tensor.matmul(scs[:], asV[:, k:k + 1], ts[:, k * S:(k + 1) * S],
                                     start=(k == 0), stop=(k == 3))
                ew = dec.tile([1, NPOS], F32, name="dew", bufs=2)
                nc.scalar.activation(ew[:], scw[:], ACT.Exp)
                es = dec.tile([1, S], F32, name="des", bufs=2)
                ses = dec.tile([1, 1], F32, name="dses", bufs=2)
                nc.scalar.activation(es[:], scs[:], ACT.Exp, accum_out=ses[:])
                sw = dec.tile([1, S], F32, name="dsw", bufs=2)
                nc.vector.tensor_reduce(sw[:], ew[:].rearrange("p (s w) -> p s w", s=S),
                                        axis=AX.X, op=ALU.add)
                rw = dec.tile([1, S], F32, name="drw", bufs=2)
                nc.vector.reciprocal(rw[:], sw[:])
                rs = dec.tile([1, 1], F32, name="drs", bufs=2)
                nc.vector.reciprocal(rs[:], ses[:])
                fs = dec.tile([1, S], F32, name="dfs", bufs=2)
                nc.vector.tensor_mul(fs[:], es[:], rw[:])
                nc.vector.tensor_scalar(fs[:], fs[:], rs[:], None, ALU.mult)
                fT = pe_t(fs[:], 1, 16)
                fTs = dec.tile([16, 1], F32, name="dfTs", bufs=2)
                nc.vector.tensor_copy(fTs[:], fT[:])
                ffull = psB.tile([1, NPOS], F32, tag="b", name="dffull")
                nc.tensor.matmul(ffull[:], fTs[:], Esb2[:], start=True, stop=True)
                pa = dec.tile([1, NPOS], F32, name="dpa", bufs=2)
                nc.vector.tensor_mul(pa[:], ew[:], ffull[:])
                paT = dec.tile([128, 4], F32, name="dpaT", bufs=2)
                for k in range(4):
                    ps = pe_t(pa[0:1, k * 128:(k + 1) * 128], 1, 128)
                    nc.vector.tensor_copy(paT[:, k:k + 1], ps[:])
                ctxT = dec.tile([128, 4], F32, name="dctxT", bufs=2)
                cps = psA.tile([128, 4], F32, tag="a", name="cps")
                for m in range(4):
                    for k in range(4):
                        nc.tensor.matmul(cps[:, m:m + 1],
                                         whR[:, k * D + m * 128: k * D + (m + 1) * 128],
                                         paT[:, k:k + 1], start=(k == 0), stop=(k == 3))
                nc.vector.tensor_copy(ctxT[:], cps[:])
                # gi_ctx half: gC [128,12]
                gC = psC.tile([128, 12], F32, tag="c2", name="gC")
                for m in range(12):
                    for k in range(4):
                        nc.tensor.matmul(
                            gC[:, m:m + 1],
                            dWc[:, k * 1536 + m * 128: k * 1536 + (m + 1) * 128],
                            ctxT[:, k:k + 1], start=(k == 0), stop=(k == 3))
                # gates
                rzp = dec.tile([128, 8], F32, name="drzp", bufs=2)
                nc.vector.tensor_add(
                    rzp[:].rearrange("p (m o) -> p m o", m=8),
                    gH[:, 0:8].rearrange("p (m o) -> p m o", m=8),
                    giov[:, 0:8, t:t + 1])
                nc.vector.tensor_add(rzp[:], rzp[:], gC[:, 0:8])
                rz = dec.tile([128, 8], F32, name="drz", bufs=2)
                nc.scalar.activation(rz[:], rzp[:], ACT.Tanh, scale=0.5)
                nc.scalar.activation(rz[:], rz[:], ACT.Identity, bias=0.5, scale=0.5)
                npre = dec.tile([128, 4], F32, name="dnpre", bufs=2)
                nc.vector.tensor_mul(npre[:], rz[:, 0:4], gH[:, 8:12])
                nc.vector.tensor_add(npre[:], npre[:], gC[:, 8:12])
                nc.vector.tensor_add(
                    npre[:].rearrange("p (m o) -> p m o", m=4),
                    npre[:].rearrange("p (m o) -> p m o", m=4),
                    giov[:, 8:12, t:t + 1])
                nn_ = dec.tile([128, 4], F32, name="dnn", bufs=2)
                nc.scalar.activation(nn_[:], npre[:], ACT.Tanh)
                dd = dec.tile([128, 4], F32, name="ddd", bufs=2)
                nc.vector.tensor_sub(dd[:], hT[:], nn_[:])
                nc.vector.tensor_mul(dd[:], rz[:, 4:8], dd[:])
                h2 = dec.tile([128, 4], F32, name="dh2", bufs=2)
                nc.vector.tensor_add(h2[:], nn_[:], dd[:])
                # pointer gate (off critical path)
                t1 = psB.tile([1, 512], F32, tag="b", name="dt1")
                for k in range(4):
                    nc.tensor.matmul(t1[:], h2[:, k:k + 1], selWT[:, k * 512:(k + 1) * 512],
                                     start=(k == 0), stop=False)
                for k in range(4):
                    nc.tensor.matmul(t1[:], OTs[:, k * ST + t: k * ST + t + 1],
                                     selWT[:, (4 + k) * 512:(5 + k) * 512],
                                     start=False, stop=False)
                for k in range(4):
                    nc.tensor.matmul(t1[:], ctxT[:, k:k + 1],
                                     selWT[:, (8 + k) * 512:(9 + k) * 512],
                                     start=False, stop=False)
                nc.tensor.matmul(t1[:], ones1[:], selB[:], start=False, stop=True)
                t2 = dec.tile([1, 512], F32, name="dt2", bufs=2)
                nc.scalar.activation(t2[:], t1[:], ACT.Tanh)
                nc.vector.tensor_mul(t2[:], t2[:], vsW[:])
                pgl = dec.tile([1, 1], F32, name="dpgl", bufs=2)
                nc.vector.tensor_reduce(pgl[:], t2[:], axis=AX.X, op=ALU.add)
                nc.vector.tensor_add(pgl[:], pgl[:], vsb[:])
                pg = dec.tile([1, 1], F32, name="dpg", bufs=2)
                nc.scalar.activation(pg[:], pgl[:], ACT.Tanh, scale=0.5)
                nc.scalar.activation(pg[:], pg[:], ACT.Identity, bias=0.5, scale=0.5)
                nc.vector.tensor_copy(PGf[0:1, t:t + 1], pg[:])
                opg = dec.tile([1, 1], F32, name="dopg", bufs=2)
                nc.scalar.activation(opg[:], pg[:], ACT.Identity, bias=1.0, scale=-1.0)
                # broadcast (1-pg) to all partitions, scale pa^T, store col t
                ob128 = psA.tile([128, 1], F32, tag="a", name="ob128")
                nc.tensor.matmul(ob128[:], ones128r[:], opg[:], start=True, stop=True)
                ob128s = dec.tile([128, 1], F32, name="ob128s", bufs=2)
                nc.vector.tensor_copy(ob128s[:], ob128[:])
                nc.vector.tensor_scalar(
                    PAT[:].rearrange("p (k t) -> p k t", k=4)[:, :, t:t + 1],
                    paT[:].rearrange("p (k o) -> p k o", k=4),
                    ob128s[:], None, ALU.mult)
                # commit state
                nc.vector.tensor_copy(hT[:], h2[:])
                nc.vector.tensor_copy(H2T3[:, :, t:t + 1],
                                      h2[:].rearrange("p (k o) -> p k o", k=4))

        # ==================== PHASE B ====================
        with tc.tile_pool(name="pb", bufs=1) as pb, \
             tc.tile_pool(name="obp", bufs=3) as obp:
            H2b = pb.tile([128, 5 * ST], BF16, name="H2b")
            for k in range(4):
                nc.vector.tensor_copy(H2b[:, k * ST:(k + 1) * ST], H2T[:, k * ST:(k + 1) * ST])
            nc.vector.memset(H2b[:, 4 * ST:5 * ST], 0.0)
            nc.vector.memset(H2b[0:1, 4 * ST:5 * ST], 1.0)
            oBs = pb.tile([1, VPAD], BF16, name="oBs")
            nc.sync.dma_start(out=oBs[:], in_=oB_d[:])
            E2d = pb.tile([128, 16 * 512], BF16, name="E2d")
            dens = pb.tile([ST, 64], F32, name="dens")

            oWv = oWT_d[:].rearrange("k p n -> p k n")
            for c in range(NCH):
                ob = obp.tile([128, 4 * 512], BF16, name="ob")
                nc.sync.dma_start(out=ob[:].rearrange("p (k n) -> p k n", k=4),
                                  in_=oWv[:, :, c * 512:(c + 1) * 512])
                lg = psB.tile([ST, 512], F32, tag="b", name="lg")
                for k in range(4):
                    nc.tensor.matmul(lg[:], H2b[:, k * ST:(k + 1) * ST],
                                     ob[:, k * 512:(k + 1) * 512],
                                     start=(k == 0), stop=False)
                nc.tensor.matmul(lg[:], H2b[0:1, 4 * ST:5 * ST],
                                 oBs[0:1, c * 512:(c + 1) * 512], start=False, stop=True)
                pr, pc = (c % 4) * 32, (c // 4) * 512
                nc.scalar.activation(E2d[pr:pr + 32, pc:pc + 512], lg[:], ACT.Exp,
                                     accum_out=dens[:, c:c + 1])
            den = pb.tile([ST, 1], F32, name="den")
            nc.vector.tensor_reduce(den[:], dens[:, 0:NCH], axis=AX.X, op=ALU.add)
            rden = pb.tile([ST, 1], F32, name="rden")
            nc.vector.reciprocal(rden[:], den[:])
            pgc_ps = pe_t(PGf[0:1, 0:ST], 1, ST)
            pgc = pb.tile([ST, 1], F32, name="pgc")
            nc.vector.tensor_copy(pgc[:], pgc_ps[:])
            fac = pb.tile([ST, 1], F32, name="fac")
            nc.vector.tensor_mul(fac[:], pgc[:], rden[:])
            for c in range(NCH):
                if c * 512 >= V:
                    break
                ncols = min(512, V - c * 512)
                so = obp.tile([ST, 512], F32, name="so")
                pr, pc = (c % 4) * 32, (c // 4) * 512
                nc.vector.tensor_scalar(so[:, 0:ncols], E2d[pr:pr + 32, pc:pc + ncols],
                                        fac[:], None, ALU.mult)
                nc.sync.dma_start(out=dense_d[:, c * 512: c * 512 + ncols],
                                  in_=so[:, 0:ncols])

            # pointer scatter-add
            Gt = pb.tile([128, 4 * 512], F32, name="Gt")
            for k in range(4):
                nc.sync.dma_start(out=Gt[:, k * 512:(k + 1) * 512], in_=G_d[k])
            sidx = pb.tile([128, 32], I16, name="sidx")
            nc.sync.dma_start(out=sidx[:], in_=SIDX_d[:])
            SCT = pb.tile([128, 4 * 64], F32, name="SCT")
            for m in range(4):
                ps = psA.tile([128, 64], F32, tag="a")
                for k in range(4):
                    nc.tensor.matmul(ps[:], Gt[:, k * 512 + m * 128: k * 512 + (m + 1) * 128],
                                     PAT[:, k * 64:(k + 1) * 64],
                                     start=(k == 0), stop=(k == 3))
                nc.vector.tensor_copy(SCT[:, m * 64:(m + 1) * 64], ps[:])
            nc.gpsimd.dma_scatter_add(
                scat_d[:], SCT[:].rearrange("p (k e) -> p k e", k=4), sidx[:],
                512, 512, 64)

    nc.finalize()
    return nc


# ----------------------------------------------------------------------------
# host side
# ----------------------------------------------------------------------------

def kernel(**inputs):
    global LAST_RESULTS
    f32 = np.float32
    bf16 = ml_dtypes.bfloat16
    p = inputs["params"]
    input_ids = np.asarray(inputs["input_ids"])
    pos_ids = np.asarray(inputs["pos_ids"])
    ner_ids = np.asarray(inputs["ner_ids"])
    tfidf_ids = np.asarray(inputs["tfidf_ids"])
    labels = np.asarray(inputs["labels"])
    extra_zeros = np.asarray(inputs["extra_zeros"], f32)
    ebev = np.asarray(inputs["enc_batch_extend_vocab"])

    def gmat(g):
        Wih = np.asarray(g["Wih"], f32); Whh = np.asarray(g["Whh"], f32)
        bih = np.asarray(g["bih"], f32); bhh = np.asarray(g["bhh"], f32)
        nh = Whh.shape[0] // 3
        bias = bih.copy()
        bias[:2 * nh] += bhh[:2 * nh]
        assert np.allclose(bhh[2 * nh:], 0.0), "nonzero bhh_n unsupported"
        return np.ascontiguousarray(Wih.T), np.ascontiguousarray(Whh.T), bias

    wf_WT, wf_UT, wf_b = gmat(p["wrnn_f"])
    wb_WT, wb_UT, wb_b = gmat(p["wrnn_b"])
    sf_WT, sf_UT, sf_b = gmat(p["srnn_f"])
    sb_WT, sb_UT, sb_b = gmat(p["srnn_b"])
    d_WT, d_UT, d_b = gmat(p["dgru"])

    def ktile(a, kt, part=128):
        rows, cols = a.shape
        out = np.zeros((kt * part, cols), f32)
        out[:rows] = a
        return np.ascontiguousarray(out.reshape(kt, part, cols))

    def coltiles(v, kt=4, part=128):
        out = np.zeros((kt * part,), f32)
        out[:v.shape[0]] = v
        return np.ascontiguousarray(out.reshape(kt, part).T)

    def aug_w(WT, b):
        out = np.zeros((256, 768), f32)
        out[:224] = WT
        out[224] = b
        return ktile(out, 2)

    shared = dict(
        wWf=aug_w(wf_WT, wf_b), wWb=aug_w(wb_WT, wb_b),
        wUf=ktile(wf_UT, 2), wUb=ktile(wb_UT, 2),
        sWf=ktile(sf_WT, 4), sWb=ktile(sb_WT, 4),
        sBf=sf_b.reshape(1, 768), sBb=sb_b.reshape(1, 768),
        sUf=ktile(sf_UT, 2), sUb=ktile(sb_UT, 2),
        waW=coltiles(np.asarray(p["wa_w"], f32)),
        aW1T=ktile(np.asarray(p["aw_W1"], f32).T.copy(), 4),
        aW2T=ktile(np.asarray(p["aw_W2"], f32).T.copy(), 4),
        sW1T=ktile(np.asarray(p["as_W1"], f32).T.copy(), 4),
        sW2T=ktile(np.asarray(p["as_W2"], f32).T.copy(), 4),
        aC=coltiles(np.asarray(p["aw_b1"], f32) + np.asarray(p["aw_b2"], f32)),
        sC=coltiles(np.asarray(p["as_b1"], f32) + np.asarray(p["as_b2"], f32)),
        awV=coltiles(np.asarray(p["aw_V"], f32)),
        asV=coltiles(np.asarray(p["as_V"], f32)),
        Esb=np.kron(np.eye(16, dtype=f32), np.ones((1, 32), f32)),
        speT=ktile(np.asarray(p["sent_pos_emb"], f32)[:16].T.copy(), 4),
        dWo=ktile(d_WT[:512], 4), dWc=ktile(d_WT[512:], 4), dU=ktile(d_UT, 4),
        dB=d_b.reshape(1, 1536),
        selWT=ktile(np.asarray(p["sel_w"], f32).T.copy(), 12),
        selB=np.asarray(p["sel_b"], f32).reshape(1, 512),
        vsW=np.asarray(p["vs_w"], f32).reshape(1, 512),
        vsb=np.asarray(p["vs_b"], f32).reshape(1, 1),
        I128=np.eye(128, dtype=f32),
    )
    oWT_full = np.zeros((512, VPAD), f32)
    oWT_full[:, :V] = np.asarray(p["out_w"], f32).T
    shared["oWT"] = np.ascontiguousarray(oWT_full.reshape(4, 128, VPAD)).astype(bf16)
    oB_full = np.full((1, VPAD), -1e4, f32)
    oB_full[0, :V] = np.asarray(p["out_b"], f32)
    shared["oB"] = oB_full.astype(bf16)

    word_emb = np.asarray(p["word_emb"], f32)
    pos_emb = np.asarray(p["pos_emb"], f32)
    ner_emb = np.asarray(p["ner_emb"], f32)
    tfidf_emb = np.asarray(p["tfidf_emb"], f32)
    dec_emb = np.asarray(p["dec_emb"], f32)

    in_maps = []
    for b in range(B):
        safe = np.where(input_ids[b] >= V, 2, np.maximum(input_ids[b], 0))
        X = np.concatenate([
            word_emb[safe],
            pos_emb[np.clip(pos_ids[b], 0, 31)],
            ner_emb[np.clip(ner_ids[b], 0, 15)],
            tfidf_emb[np.clip(tfidf_ids[b], 0, 9)],
        ], axis=-1)
        Xw = np.transpose(X, (1, 0, 2)).reshape(NPOS, 224)   # w-major tokens
        xTa = np.zeros((256, NPOS), f32)
        xTa[:224] = Xw.T
        xTa[224] = 1.0

        prev = np.concatenate([[BOS], labels[b, :-1]]).astype(np.int64)
        sp = np.where(prev >= V, 2, np.maximum(prev, 0))
        O = np.maximum(dec_emb[sp], 0.0)
        OT_h = ktile(np.ascontiguousarray(O.T), 4)

        idx = ebev[b].reshape(NPOS).astype(np.int64)
        idx = np.where(idx >= VEXT, 0, idx)
        uniq, inv = np.unique(idx, return_inverse=True)
        m = uniq.shape[0]
        sflat = np.zeros(NPOS, np.int16)
        sflat[:m] = uniq.astype(np.int16)
        SIDX_h = np.zeros((16, 32), np.int16)
        for i in range(NPOS):
            SIDX_h[i % 16, i // 16] = sflat[i]
        SIDX_h = np.tile(SIDX_h, (8, 1))
        Gm = np.zeros((NPOS, NPOS), f32)
        Gm[np.arange(NPOS), inv] = 1.0

        im = dict(shared)
        im.update(xTa=ktile(xTa, 2), OT=OT_h, SIDX=SIDX_h, G=ktile(Gm, 4))
        in_maps.append(im)

    nc = build_nc()
    res = run_bass_kernel_spmd(nc, in_maps, core_ids=list(range(B)))
    LAST_RESULTS = res

    out = np.zeros((B, ST, VEXT), np.float32)
    for b in range(B):
        dense = np.asarray(res.results[b]["dense"], np.float32)
        scat = np.asarray(res.results[b]["scat"], np.float32)
        out[b, :, :V] = dense
        out[b, :, V:] = extra_zeros[b][None, :]
        out[b] += scat[:, :ST].T
    return out
